# revision 15
# baseline (speedup 1.0000x reference)
"""Trainium2 Bass kernel for nn_BandPassFilter (filtfilt FIR bank).

Math: the reference does, per band n, a 'same' cross-correlation with w[n]
followed by flip/conv/flip (filtfilt), over an odd-extended signal, then
crops padlen=2307 from each side.  Composing the two passes, each band's
combined filter is the autocorrelation c[n] = corr(w[n], w[n]) of length
2K-1 = 1537, and since padlen > 2*(K-1) the cropped region never touches
the conv zero-padding.  So:

    out[b, n, t] = sum_{k=0}^{1536} c[n, k] * xext[b, t + k]

with xext = [-flip(xs[1:769]), xs, -flip(xs[-769:-1])], length 9728.

Mapping to the 128x128 PE array (per core, 8 batch rows):
  - stationary lhsT[p, (r2, f')] = xext[r, 128*(f'+ci) + p]  -- a plain
    column-major view xq[p, r, q] = xext[r, 128 q + p], no replication.
  - moving rhs[p, (n, m2)] = c[n, 128*ci + p - m2]  -- host-precomputed
    Toeplitz bank cb, 13 contraction chunks ci cover k in [0, 1664).
  - PSUM out[(r2, f'), (n, m2)] = out[r, n, 128 f' + m2] accumulates the
    13 chunks; per-partition 512B-contiguous runs -> clean output DMA.

Sharding: data-parallel over batch, 8 rows per NeuronCore, kernels
replicated (per the band-parallel-free variant of the hint).
"""
import numpy as np

B, L, NB, K = 64, 8192, 20, 769
KC = 2 * K - 1      # 1537 combined filter length
PAD = K - 1         # 768
LE = L + 2 * PAD    # 9728 = 128 * 76
QCOLS = LE // 128   # 76 = 64 + 12
NCH = 13            # 13*128 = 1664 >= KC + 127
NCORES = 8
RPC = B // NCORES   # 8 rows per core
NBG = 5             # 5 groups of 4 bands
CSCALE = 1024.0     # keep fp16 c values out of the subnormal range

_CACHE = {}


def _program(dt_name):
    import concourse.bass as bass
    import concourse.bacc as bacc
    import concourse.tile as tile
    from concourse import mybir

    DT = getattr(mybir.dt, dt_name)
    f32 = mybir.dt.float32
    nc = bacc.Bacc()
    xq_d = nc.dram_tensor("xq", [128, RPC, QCOLS], DT, kind="ExternalInput")
    cb_d = nc.dram_tensor("cb", [128, NBG, NCH, 4, 128], DT,
                          kind="ExternalInput")
    out_d = nc.dram_tensor("out", [RPC, NB, L // 128, 128], f32,
                           kind="ExternalOutput")
    with tile.TileContext(nc) as tc:
        with (
            tc.tile_pool(name="xqp", bufs=1) as xqp,
            tc.tile_pool(name="cbp", bufs=2) as cbp,
            tc.tile_pool(name="stp", bufs=NBG * 4) as stp,
            tc.tile_pool(name="psp", bufs=4, space=bass.MemorySpace.PSUM) as psp,
        ):
            # dst view so one DMA per (g, rp): dims (r2, f, n, m)
            out_v = out_d[:].rearrange("r n f m -> r f n m")
            xq_t = xqp.tile([128, RPC, QCOLS], DT)
            nc.sync.dma_start(xq_t[:], xq_d[:])
            for g in range(NBG):
                cbt = cbp.tile([128, NCH, 4, 128], DT)
                nc.sync.dma_start(cbt[:], cb_d[:, g])
                # dummy weight load: absorbs the cb-DMA wait onto the PE
                # queue so group-leader matmuls stay within 2 wait slots
                nc.tensor.ldweights(cbt[:, 0, 0])
                for rp in range(RPC // 2):
                    ps = psp.tile([128, 4, 128], f32)
                    for ci in range(NCH):
                        for r2 in range(2):
                            # weights need one free dim -> col-tile the two
                            # rows onto separate 64-col halves of the array
                            nc.tensor.matmul(
                                ps[r2 * 64:(r2 + 1) * 64],
                                xq_t[:, rp * 2 + r2, ci:ci + 64],
                                cbt[:, ci],
                                start=(ci == 0),
                                stop=(ci == NCH - 1),
                                tile_position=(0, 64 * r2),
                            )
                    for r2 in range(2):
                        st = stp.tile([64, 4, 128], f32, tag=f"st{r2}")
                        nc.scalar.mul(st[:], ps[r2 * 64:(r2 + 1) * 64],
                                      1.0 / CSCALE)
                        nc.sync.dma_start(
                            out_v[rp * 2 + r2, :, g * 4:(g + 1) * 4], st[:])
    nc.compile()
    return nc


def _prep(x, kernels, np_dt):
    xs = np.asarray(x)[:, 0, :].astype(np.float32)
    w = np.asarray(kernels).astype(np.float32)
    xext = np.concatenate(
        [-xs[:, PAD:0:-1], xs, -xs[:, L - 2:L - 2 - PAD:-1]], axis=1)
    xq = xext.reshape(B, QCOLS, 128)  # [B, q, p]
    # per-core [128, RPC, QCOLS]
    xq_cores = [
        np.ascontiguousarray(
            xq[c * RPC:(c + 1) * RPC].transpose(2, 0, 1).astype(np_dt))
        for c in range(NCORES)
    ]
    c = np.stack([np.correlate(w[n], w[n], "full") for n in range(NB)])
    idx = (128 * np.arange(NCH))[None, :, None] \
        + np.arange(128)[:, None, None] - np.arange(128)[None, None, :]
    valid = (idx >= 0) & (idx < KC)
    cb = np.where(valid[None], c[:, np.clip(idx, 0, KC - 1)] * CSCALE, 0.0)
    # cb: [NB, 128, NCH, 128] -> [128, NBG, NCH, 4, 128]
    cb = cb.reshape(NBG, 4, 128, NCH, 128).transpose(2, 0, 3, 1, 4)
    cb = np.ascontiguousarray(cb.astype(np_dt))
    return xq_cores, cb


def _run(x, kernels, dt_name="float16", np_dt=np.float16, **run_kwargs):
    from concourse.bass_utils import run_bass_kernel_spmd

    key = dt_name
    if key not in _CACHE:
        _CACHE[key] = _program(dt_name)
    nc = _CACHE[key]
    xq_cores, cb = _prep(x, kernels, np_dt)
    in_maps = [{"xq": xq_cores[c], "cb": cb} for c in range(NCORES)]
    res = run_bass_kernel_spmd(nc, in_maps, core_ids=list(range(NCORES)),
                               **run_kwargs)
    out = np.concatenate(
        [res.results[c]["out"].reshape(RPC, NB, L) for c in range(NCORES)],
        axis=0)
    return out[:, None].astype(np.float32), res


def kernel(x, kernels):
    out, _ = _run(x, kernels)
    return out


# revision 24
# speedup vs baseline: 270.4974x; 270.4974x over previous
"""Trainium2 Bass kernel for nn_BandPassFilter (filtfilt FIR bank).

Math: the reference does, per band n, a 'same' cross-correlation with w[n]
followed by flip/conv/flip (filtfilt), over an odd-extended signal, then
crops padlen=2307 from each side.  Composing the two passes, each band's
combined filter is the autocorrelation c[n] = corr(w[n], w[n]) of length
2K-1 = 1537, and since padlen > 2*(K-1) the cropped region never touches
the conv zero-padding.  So:

    out[b, n, t] = sum_{k=0}^{1536} c[n, k] * xext[b, t + k]

with xext = [-flip(xs[1:769]), xs, -flip(xs[-769:-1])], length 9728.

Mapping to the 128x128 PE array (per core, 8 batch rows):
  - stationary lhsT[p, 2f'+r2] = xext[2rp+r2, 128*(f'+ci) + p] -- a plain
    column-major view with the two rows of a pair interleaved in q:
    xq2[p, rp, 2q+r2] = xext[2rp+r2, 128q+p], so each (rp, ci) weight
    slice xq2[:, rp, 2ci:2ci+128] is one single-stride 128-column load.
  - moving rhs[p, (n, m2)] = c[n, 128*ci + p - m2]  -- host-precomputed
    Toeplitz bank cb, 13 contraction chunks ci cover k in [0, 1664).
  - PSUM out[(r2, f'), (n, m2)] = out[r, n, 128 f' + m2] accumulates the
    13 chunks; per-partition 512B-contiguous runs -> clean output DMA.

Sharding: data-parallel over batch, 8 rows per NeuronCore, kernels
replicated (per the band-parallel-free variant of the hint).
"""
import numpy as np

B, L, NB, K = 64, 8192, 20, 769
KC = 2 * K - 1      # 1537 combined filter length
PAD = K - 1         # 768
LE = L + 2 * PAD    # 9728 = 128 * 76
QCOLS = LE // 128   # 76 = 64 + 12
NCH = 13            # 13*128 = 1664 >= KC + 127
NCORES = 8
RPC = B // NCORES   # 8 rows per core
NBG = 5             # 5 groups of 4 bands
CSCALE = 1024.0     # keep fp16 c values out of the subnormal range

_CACHE = {}


def _program(dt_name):
    import concourse.bass as bass
    import concourse.bacc as bacc
    import concourse.tile as tile
    from concourse import mybir

    DT = getattr(mybir.dt, dt_name)
    f32 = mybir.dt.float32
    nc = bacc.Bacc()
    xq_d = nc.dram_tensor("xq", [128, RPC // 2, 2 * QCOLS], DT,
                          kind="ExternalInput")
    cb_d = nc.dram_tensor("cb", [128, NBG, NCH, 4, 128], DT,
                          kind="ExternalInput")
    out_d = nc.dram_tensor("out", [RPC, NB, L // 128, 128], f32,
                           kind="ExternalOutput")
    with tile.TileContext(nc) as tc:
        with (
            tc.tile_pool(name="xqp", bufs=1) as xqp,
            tc.tile_pool(name="cbp", bufs=3) as cbp,
            tc.tile_pool(name="stp", bufs=NBG * 4) as stp,
            tc.tile_pool(name="psp", bufs=4, space=bass.MemorySpace.PSUM) as psp,
        ):
            # dst view: per (row, band-group) dims (f, n, m)
            out_v = out_d[:].rearrange("r n f m -> r f n m")
            xq_t = xqp.tile([128, RPC // 2, 2 * QCOLS], DT)
            # prologue: first matmul needs only cb[g0,ci0] + xq[rp0]; land
            # them on the two HWDGE rings in parallel, then batch the rest
            # in growing chunks so delivery outpaces PE consumption
            cbt0 = cbp.tile([128, NCH, 4, 128], DT)
            nc.scalar.dma_start(xq_t[:], xq_d[:])
            nc.sync.dma_start(cbt0[:, 0], cb_d[:, 0, 0])
            for lo, hi in ((1, 3), (3, 7), (7, NCH)):
                nc.sync.dma_start(cbt0[:, lo:hi], cb_d[:, 0, lo:hi])
            for g in range(NBG):
                if g == 0:
                    cbt = cbt0
                else:
                    cbt = cbp.tile([128, NCH, 4, 128], DT)
                    nc.sync.dma_start(cbt[:], cb_d[:, g])
                # dummy weight load: absorbs the cb-DMA wait onto the PE
                # queue so group-leader matmuls stay within 2 wait slots
                nc.tensor.ldweights(cbt[:, 0, 0])
                nrp = RPC // 2
                pss = [psp.tile([128, 4, 128], f32, name=f"ps{g}_{i}",
                                tag="ps") for i in range(nrp)]
                if g == 0:
                    # ci-outer: 4 matmuls of PE work per arriving cb chunk,
                    # so the prologue trickle-DMA keeps the PE fed
                    for ci in range(NCH):
                        for rp in range(nrp):
                            nc.tensor.matmul(
                                pss[rp][:],
                                xq_t[:, rp, 2 * ci:2 * ci + 128],
                                cbt[:, ci],
                                start=(ci == 0), stop=(ci == NCH - 1),
                            )
                else:
                    for rp in range(nrp):
                        for ci in range(NCH):
                            nc.tensor.matmul(
                                pss[rp][:],
                                xq_t[:, rp, 2 * ci:2 * ci + 128],
                                cbt[:, ci],
                                start=(ci == 0), stop=(ci == NCH - 1),
                            )
                for rp in range(nrp):
                    st = stp.tile([128, 4, 128], f32)
                    nc.scalar.mul(st[:], pss[rp][:], 1.0 / CSCALE)
                    for r2 in range(2):
                        # psum partition m = 2f' + r2 -> every-other partition
                        nc.sync.dma_start(
                            out_v[rp * 2 + r2, :, g * 4:(g + 1) * 4],
                            st[r2::2])
    nc.compile()
    return nc


def _prep(x, kernels, np_dt):
    xs = np.asarray(x)[:, 0, :].astype(np.float32)
    w = np.asarray(kernels).astype(np.float32)
    xext = np.concatenate(
        [-xs[:, PAD:0:-1], xs, -xs[:, L - 2:L - 2 - PAD:-1]], axis=1)
    # per-core [128, RPC//2, 2*QCOLS]: rows of a pair interleaved in q
    xq_cores = [
        np.ascontiguousarray(
            xext[c * RPC:(c + 1) * RPC]
            .reshape(RPC // 2, 2, QCOLS, 128)
            .transpose(3, 0, 2, 1)
            .reshape(128, RPC // 2, 2 * QCOLS)
            .astype(np_dt))
        for c in range(NCORES)
    ]
    c = np.stack([np.correlate(w[n], w[n], "full") for n in range(NB)])
    idx = (128 * np.arange(NCH))[None, :, None] \
        + np.arange(128)[:, None, None] - np.arange(128)[None, None, :]
    valid = (idx >= 0) & (idx < KC)
    cb = np.where(valid[None], c[:, np.clip(idx, 0, KC - 1)] * CSCALE, 0.0)
    # cb: [NB, 128, NCH, 128] -> [128, NBG, NCH, 4, 128]
    cb = cb.reshape(NBG, 4, 128, NCH, 128).transpose(2, 0, 3, 1, 4)
    cb = np.ascontiguousarray(cb.astype(np_dt))
    return xq_cores, cb


def _run(x, kernels, dt_name="float16", np_dt=np.float16, **run_kwargs):
    from concourse.bass_utils import run_bass_kernel_spmd

    key = dt_name
    if key not in _CACHE:
        _CACHE[key] = _program(dt_name)
    nc = _CACHE[key]
    xq_cores, cb = _prep(x, kernels, np_dt)
    in_maps = [{"xq": xq_cores[c], "cb": cb} for c in range(NCORES)]
    res = run_bass_kernel_spmd(nc, in_maps, core_ids=list(range(NCORES)),
                               **run_kwargs)
    out = np.concatenate(
        [res.results[c]["out"].reshape(RPC, NB, L) for c in range(NCORES)],
        axis=0)
    return out[:, None].astype(np.float32), res


def kernel(x, kernels):
    out, _ = _run(x, kernels)
    return out


# revision 25
# speedup vs baseline: 1269.3648x; 4.6927x over previous
"""Trainium2 Bass kernel for nn_BandPassFilter (filtfilt FIR bank).

Math: the reference does, per band n, a 'same' cross-correlation with w[n]
followed by flip/conv/flip (filtfilt), over an odd-extended signal, then
crops padlen=2307 from each side.  Composing the two passes, each band's
combined filter is the autocorrelation c[n] = corr(w[n], w[n]) of length
2K-1 = 1537, and since padlen > 2*(K-1) the cropped region never touches
the conv zero-padding.  So:

    out[b, n, t] = sum_{k=0}^{1536} c[n, k] * xext[b, t + k]

with xext = [-flip(xs[1:769]), xs, -flip(xs[-769:-1])], length 9728.

Mapping to the 128x128 PE array (per core, 8 batch rows):
  - stationary lhsT[p, 2f'+r2] = xext[2rp+r2, 128*(f'+ci) + p] -- a plain
    column-major view with the two rows of a pair interleaved in q:
    xq2[p, rp, 2q+r2] = xext[2rp+r2, 128q+p], so each (rp, ci) weight
    slice xq2[:, rp, 2ci:2ci+128] is one single-stride 128-column load.
  - moving rhs[p, (n, m2)] = c[n, 128*ci + p - m2]  -- host-precomputed
    Toeplitz bank cb, 13 contraction chunks ci cover k in [0, 1664).
  - PSUM out[(r2, f'), (n, m2)] = out[r, n, 128 f' + m2] accumulates the
    13 chunks; per-partition 512B-contiguous runs -> clean output DMA.

Sharding: data-parallel over batch, 8 rows per NeuronCore, kernels
replicated (per the band-parallel-free variant of the hint).
"""
import numpy as np

B, L, NB, K = 64, 8192, 20, 769
KC = 2 * K - 1      # 1537 combined filter length
PAD = K - 1         # 768
LE = L + 2 * PAD    # 9728 = 128 * 76
QCOLS = LE // 128   # 76 = 64 + 12
NCH = 13            # 13*128 = 1664 >= KC + 127
NCORES = 8
RPC = B // NCORES   # 8 rows per core
NBG = 5             # 5 groups of 4 bands
CSCALE = 1024.0     # keep fp16 c values out of the subnormal range

_CACHE = {}


def _program(dt_name):
    import concourse.bass as bass
    import concourse.bacc as bacc
    import concourse.tile as tile
    from concourse import mybir

    DT = getattr(mybir.dt, dt_name)
    f32 = mybir.dt.float32
    nc = bacc.Bacc()
    xq_d = nc.dram_tensor("xq", [128, RPC // 2, 2 * QCOLS], DT,
                          kind="ExternalInput")
    cb_d = nc.dram_tensor("cb", [128, NBG, NCH, 4, 128], DT,
                          kind="ExternalInput")
    out_d = nc.dram_tensor("out", [RPC, NB, L // 128, 128], f32,
                           kind="ExternalOutput")
    with tile.TileContext(nc) as tc:
        with (
            tc.tile_pool(name="xqp", bufs=1) as xqp,
            tc.tile_pool(name="cbp", bufs=3) as cbp,
            tc.tile_pool(name="stp", bufs=NBG * 4) as stp,
            tc.tile_pool(name="psp", bufs=4, space=bass.MemorySpace.PSUM) as psp,
        ):
            # dst view: per (row, band-group) dims (f, n, m)
            out_v = out_d[:].rearrange("r n f m -> r f n m")
            xq_t = xqp.tile([128, RPC // 2, 2 * QCOLS], DT)
            # prologue: first matmul needs only cb[g0,ci0] + xq[rp0]; land
            # them on the two HWDGE rings in parallel, then batch the rest
            # in growing chunks so delivery outpaces PE consumption
            cbt0 = cbp.tile([128, NCH, 4, 128], DT)
            nc.scalar.dma_start(xq_t[:], xq_d[:])
            nc.sync.dma_start(cbt0[:, 0], cb_d[:, 0, 0])
            for lo, hi in ((1, 3), (3, 7), (7, NCH)):
                nc.sync.dma_start(cbt0[:, lo:hi], cb_d[:, 0, lo:hi])
            for g in range(NBG):
                if g == 0:
                    cbt = cbt0
                else:
                    cbt = cbp.tile([128, NCH, 4, 128], DT)
                    nc.sync.dma_start(cbt[:], cb_d[:, g])
                # dummy weight load: absorbs the cb-DMA wait onto the PE
                # queue so group-leader matmuls stay within 2 wait slots
                # (fp32/f32r ldweights is unsupported; bacc then splits via
                # event semaphores instead)
                if mybir.dt.size(DT) == 2:
                    nc.tensor.ldweights(cbt[:, 0, 0])
                nrp = RPC // 2
                pss = [psp.tile([128, 4, 128], f32, name=f"ps{g}_{i}",
                                tag="ps") for i in range(nrp)]
                if g == 0:
                    # ci-outer: 4 matmuls of PE work per arriving cb chunk,
                    # so the prologue trickle-DMA keeps the PE fed
                    for ci in range(NCH):
                        for rp in range(nrp):
                            nc.tensor.matmul(
                                pss[rp][:],
                                xq_t[:, rp, 2 * ci:2 * ci + 128],
                                cbt[:, ci],
                                start=(ci == 0), stop=(ci == NCH - 1),
                            )
                else:
                    for rp in range(nrp):
                        for ci in range(NCH):
                            nc.tensor.matmul(
                                pss[rp][:],
                                xq_t[:, rp, 2 * ci:2 * ci + 128],
                                cbt[:, ci],
                                start=(ci == 0), stop=(ci == NCH - 1),
                            )
                for rp in range(nrp):
                    st = stp.tile([128, 4, 128], f32)
                    nc.scalar.mul(st[:], pss[rp][:], 1.0 / CSCALE)
                    for r2 in range(2):
                        # psum partition m = 2f' + r2 -> every-other partition
                        nc.sync.dma_start(
                            out_v[rp * 2 + r2, :, g * 4:(g + 1) * 4],
                            st[r2::2])
    nc.compile()
    return nc


def _prep(x, kernels, np_dt):
    xs = np.asarray(x)[:, 0, :].astype(np.float32)
    w = np.asarray(kernels).astype(np.float32)
    xext = np.concatenate(
        [-xs[:, PAD:0:-1], xs, -xs[:, L - 2:L - 2 - PAD:-1]], axis=1)
    # per-core [128, RPC//2, 2*QCOLS]: rows of a pair interleaved in q
    xq_cores = [
        np.ascontiguousarray(
            xext[c * RPC:(c + 1) * RPC]
            .reshape(RPC // 2, 2, QCOLS, 128)
            .transpose(3, 0, 2, 1)
            .reshape(128, RPC // 2, 2 * QCOLS)
            .astype(np_dt))
        for c in range(NCORES)
    ]
    c = np.stack([np.correlate(w[n], w[n], "full") for n in range(NB)])
    idx = (128 * np.arange(NCH))[None, :, None] \
        + np.arange(128)[:, None, None] - np.arange(128)[None, None, :]
    valid = (idx >= 0) & (idx < KC)
    cb = np.where(valid[None], c[:, np.clip(idx, 0, KC - 1)] * CSCALE, 0.0)
    # cb: [NB, 128, NCH, 128] -> [128, NBG, NCH, 4, 128]
    cb = cb.reshape(NBG, 4, 128, NCH, 128).transpose(2, 0, 3, 1, 4)
    cb = np.ascontiguousarray(cb.astype(np_dt))
    return xq_cores, cb


def _run(x, kernels, dt_name="float16", np_dt=np.float16, **run_kwargs):
    from concourse.bass_utils import run_bass_kernel_spmd

    key = dt_name
    if key not in _CACHE:
        _CACHE[key] = _program(dt_name)
    nc = _CACHE[key]
    xq_cores, cb = _prep(x, kernels, np_dt)
    in_maps = [{"xq": xq_cores[c], "cb": cb} for c in range(NCORES)]
    res = run_bass_kernel_spmd(nc, in_maps, core_ids=list(range(NCORES)),
                               **run_kwargs)
    out = np.concatenate(
        [res.results[c]["out"].reshape(RPC, NB, L) for c in range(NCORES)],
        axis=0)
    return out[:, None].astype(np.float32), res


def kernel(x, kernels):
    out, _ = _run(x, kernels)
    return out
